# revision 23
# baseline (speedup 1.0000x reference)
"""GCN mean-aggregate + dual-encoder kernel for Trainium2 (8 NeuronCores).

Problem: h_rx = relu(W1 @ [self_feats; mean(neigh_feats)].T)
         h_rc = relu(W2 @ [self_feats; mean(neigh_feats)].T)

Sharding: batch of 20000 query nodes split 2500/core across 8 cores.
feat_data (bf16) and weights replicated.

The per-core time is floored by GPSIMD descriptor generation: every
indirect-DMA instruction costs ~1.17us on the Pool engine and carries at
most 128 row descriptors (one offset per partition; multi-column offset
APs wedge the HW DGE, and the SWDGE custom gather/scatter instructions
cost ~8ns/row with a 1024-row/instruction ring cap — both measured).
340 instructions x 1.17us ~= 400us is the floor, so everything else is
sized to hide under it:
  - feat is cast to bf16 on the host: gather traffic halves (512B
    descriptors still hit full HBM efficiency), DVE tree-sum halves,
    and the PE runs 4x faster than fp32 (1 cycle/row vs 4).
  - per 128-node tile, one indirect gather per row j (17 per tile) into
    G[128, 17*256] bf16; DVE tree-sums the 16 neighbor rows (1/16 is
    folded into the neighbor half of the weights host-side).
  - PE transposes [node, d] -> [d, node] (bf16, via identity matmul)
    feeding fused bf16 matmuls with weight [512d, 512o] = [W1 | W2];
    f32 PSUM accumulate; relu + PSUM->SBUF on DVE; store f32.

Walrus constraint: a Matmult (LDWEIGHTS struct) tolerates only one
embedded sync wait. All matmul dependencies are funneled through the
DVE semaphore: PSUM->SBUF copies and relu run on DVE, per-tile PE order
is (sum-half transposes, then self-half transposes), output tiles are
write-once, and two dummy PE warm-up ops absorb the one-time identity /
weight-load waits. Engine order is frozen with no-sync dep edges.
"""

import os

import numpy as np

N_TOTAL = 100000
B = 20000
K = 16
D = 256
OUT = 256
NCORES = 8
B_LOC = B // NCORES          # 2500
NT = 20                      # 128-node tiles per core (padded to 2560)
B_PAD = NT * 128
NG = 5                       # groups of 4 tiles (512 nodes per group)

_CACHE = {}


def _build_module():
    import concourse.bass as bass
    import concourse.mybir as mybir
    import concourse.tile as tile
    from concourse import bacc
    from concourse.tile_rust import add_dep_helper

    f32 = mybir.dt.float32
    bf16 = mybir.dt.bfloat16

    nc = bacc.Bacc("TRN2", target_bir_lowering=False)
    feat = nc.dram_tensor("feat", [N_TOTAL, D], bf16, kind="ExternalInput")
    wt = nc.dram_tensor("wt", [2 * D, 2 * OUT], bf16, kind="ExternalInput")
    ident = nc.dram_tensor("ident", [128, 128], bf16, kind="ExternalInput")
    idx = nc.dram_tensor("idx", [128, NT * 17], mybir.dt.int32, kind="ExternalInput")
    out = nc.dram_tensor("out", [2 * OUT, B_PAD], f32, kind="ExternalOutput")
    out_ap = out.ap()

    prev = {"pe": None, "dve": None}

    def chain(engine, bi):
        if prev[engine] is not None:
            add_dep_helper(bi.ins, prev[engine].ins, False, f"{engine} order")
        prev[engine] = bi
        return bi

    with tile.TileContext(nc) as tc:
        with (
            tc.tile_pool(name="const", bufs=1) as constp,
            tc.tile_pool(name="gather", bufs=6) as gp,
            tc.tile_pool(name="sums", bufs=4) as sp,
            tc.tile_pool(name="combT", bufs=2) as cp,
            tc.tile_pool(name="psum_t", bufs=4, space="PSUM") as ptp,
            tc.tile_pool(name="psum_o", bufs=3, space="PSUM") as pop,
            tc.tile_pool(name="outs", bufs=6) as op,
        ):
            # idx first: it gates the gather stream (the serial resource)
            idx_s = constp.tile([128, NT * 17], mybir.dt.int32, tag="idx")
            nc.sync.dma_start(idx_s[:], idx.ap())
            ident_s = constp.tile([128, 128], bf16, tag="ident")
            nc.sync.dma_start(ident_s[:], ident.ap())
            wt_s = constp.tile([128, 4, 2 * OUT], bf16, tag="wt")
            nc.sync.dma_start(wt_s[:], wt.ap().rearrange("(c p) o -> p c o", p=128))

            # PE warm-ups: absorb the identity / weight HWDGE semaphore
            # observations so real matmuls carry <=1 embedded wait. They
            # borrow rotation buffers; later reuse is ordered by PE chain.
            warm_t = ptp.tile([128, 128], bf16, tag="pt")
            chain("pe", nc.tensor.transpose(out=warm_t[:], in_=ident_s[:],
                                            identity=ident_s[:]))
            warm_m = pop.tile([128, 512], f32, tag="po")
            chain("pe", nc.tensor.matmul(warm_m[:, 0:128],
                                         lhsT=wt_s[:, 0, 0:128],
                                         rhs=wt_s[:, 0, 0:128],
                                         start=True, stop=True))

            def emit_tiles(t0, t1):
                combT = cp.tile([128, 4, 512], bf16)
                for t4 in range(t1 - t0):
                    t = t0 + t4
                    G = gp.tile([128, 17, D], bf16)
                    S = sp.tile([128, 16, D], bf16)
                    # one offset per partition per instruction: the HW DGE
                    # rejects multi-column offset APs (wedges the device).
                    # G is read ONLY by DVE (self-copy + pair/tree sums) so
                    # the gather's buffer-reuse WAR wait is a single DVE sem.
                    # Pair-adds interleave with the gather stream (each waits
                    # only its two rows' DMA sem values) so the per-tile DVE
                    # tail after the last gather is one pair + tree levels.
                    for j in range(17):
                        nc.gpsimd.indirect_dma_start(
                            out=G[:, j, :],
                            out_offset=None,
                            in_=feat.ap(),
                            in_offset=bass.IndirectOffsetOnAxis(
                                ap=idx_s[:, t * 17 + j:t * 17 + j + 1], axis=0
                            ),
                        )
                        if j == 0:
                            chain("dve", nc.vector.tensor_copy(
                                S[:, 15:16, :], G[:, 0:1, :]))
                        elif j % 2 == 0:
                            k = j // 2 - 1
                            chain("dve", nc.vector.tensor_add(
                                S[:, k:k + 1, :], G[:, j - 1:j, :],
                                G[:, j:j + 1, :]))
                    chain("dve", nc.vector.tensor_add(
                        S[:, 8:12, :], S[:, 0:4, :], S[:, 4:8, :]))
                    chain("dve", nc.vector.tensor_add(
                        S[:, 12:14, :], S[:, 8:10, :], S[:, 10:12, :]))
                    chain("dve", nc.vector.tensor_add(
                        S[:, 14:15, :], S[:, 12:13, :], S[:, 13:14, :]))
                    # d-chunks: 0,1 = self halves, 2,3 = sum halves, all
                    # sourced from S. Sum halves first so the self-half
                    # transposes' PSUM-WAR ticks are already observed.
                    for c in (2, 3, 0, 1):
                        if c < 2:
                            src = S[:, 15, c * 128:(c + 1) * 128]
                        else:
                            src = S[:, 14, (c - 2) * 128:(c - 1) * 128]
                        pt = ptp.tile([128, 128], bf16, tag="pt")
                        chain("pe", nc.tensor.transpose(
                            out=pt[:], in_=src, identity=ident_s[:]))
                        chain("dve", nc.vector.tensor_copy(
                            combT[:, c, t4 * 128:(t4 + 1) * 128], pt[:]))
                return combT

            def emit_mm(t0, t1, combT):
                ncols = (t1 - t0) * 128
                for oc in range(4):
                    po = pop.tile([128, 512], f32, tag="po")
                    for c in range(4):
                        chain("pe", nc.tensor.matmul(
                            po[:, 0:ncols],
                            lhsT=wt_s[:, c, oc * 128:(oc + 1) * 128],
                            rhs=combT[:, c, 0:ncols],
                            start=(c == 0),
                            stop=(c == 3),
                        ))
                    ob = op.tile([128, 512], f32, tag="ob")
                    chain("dve", nc.vector.tensor_relu(
                        ob[:, 0:ncols], po[:, 0:ncols]))
                    nc.sync.dma_start(
                        out_ap[oc * 128:(oc + 1) * 128,
                               t0 * 128:t0 * 128 + ncols],
                        ob[:, 0:ncols],
                    )

            # two-stage software pipeline: a group's matmuls are emitted
            # after the next group's gather/aggregate tiles so PE/DVE
            # overlap; the final 1-tile group keeps the drain tail short.
            groups = [(0, 4), (4, 8), (8, 12), (12, 16), (16, 19), (19, 20)]
            pending = None
            for t0, t1 in groups:
                combT = emit_tiles(t0, t1)
                if pending is not None:
                    emit_mm(*pending)
                pending = (t0, t1, combT)
            emit_mm(*pending)
    nc.compile()
    return nc


def _prep_inputs(nodes, neigh_idx, feat_data, W1, W2):
    import ml_dtypes

    bf16 = ml_dtypes.bfloat16

    feat = np.ascontiguousarray(
        np.asarray(feat_data, dtype=np.float32).astype(bf16))
    W1 = np.asarray(W1, dtype=np.float32)
    W2 = np.asarray(W2, dtype=np.float32)
    Wcat = np.concatenate([W1, W2], axis=0).copy()   # [512 o, 512 d]
    Wcat[:, D:] *= np.float32(1.0 / K)               # fold the mean's 1/K
    wt = np.ascontiguousarray(Wcat.T.astype(bf16))   # [512 d, 512 o]
    ident = np.eye(128, dtype=bf16)
    nodes = np.asarray(nodes).astype(np.int32)
    neigh = np.asarray(neigh_idx).astype(np.int32)
    in_maps = []
    for c in range(NCORES):
        lo, hi = c * B_LOC, (c + 1) * B_LOC
        blk = np.zeros((B_PAD, 17), np.int32)
        blk[:B_LOC, 0] = nodes[lo:hi]
        blk[:B_LOC, 1:] = neigh[lo:hi]
        idx_host = np.ascontiguousarray(
            blk.reshape(NT, 128, 17).transpose(1, 0, 2).reshape(128, NT * 17)
        )
        in_maps.append({"feat": feat, "wt": wt, "ident": ident,
                        "idx": idx_host})
    return in_maps


def _get_module():
    if "nc" not in _CACHE:
        _CACHE["nc"] = _build_module()
    return _CACHE["nc"]


def _axon_reset():
    try:
        import ctypes

        lib = ctypes.CDLL("/opt/axon/libaxon_pjrt.so")
        lib.axon_reset.restype = ctypes.c_int64
        lib.axon_reset()
    except Exception:
        pass


def _ensure_ntff_hook():
    """antenv.axon_hooks is absent in some images; recreate it from the boot
    shim so run_bass_kernel_spmd can trace (no-op if the real module exists)."""
    import sys
    import types

    try:
        import antenv.axon_hooks  # noqa: F401
        return
    except ImportError:
        pass
    try:
        from trn_agent_boot.trn_boot import _ntff_profile_via_ctypes
        import antenv
    except ImportError:
        return
    hook = _ntff_profile_via_ctypes("/opt/axon/libaxon_pjrt.so")
    mod = types.ModuleType("antenv.axon_hooks")
    mod.get_axon_ntff_profile_hook = lambda: hook
    mod.set_axon_ntff_profile_hook = lambda h: None
    sys.modules["antenv.axon_hooks"] = mod
    antenv.axon_hooks = mod


def kernel(nodes, neigh_idx, feat_data, W1, W2):
    from concourse.bass_utils import run_bass_kernel_spmd

    nc = _get_module()
    in_maps = _prep_inputs(nodes, neigh_idx, feat_data, W1, W2)
    _ensure_ntff_hook()
    trace = bool(int(os.environ.get("GCN_TRACE", "0")))
    try:
        res = run_bass_kernel_spmd(
            nc, in_maps, core_ids=list(range(NCORES)), trace=trace
        )
    except Exception:
        # a prior run may have left the device wedged; reset and retry once
        _axon_reset()
        res = run_bass_kernel_spmd(
            nc, in_maps, core_ids=list(range(NCORES)), trace=trace
        )
    _CACHE["last_results"] = res
    h = np.empty((2 * OUT, B), np.float32)
    for c in range(NCORES):
        h[:, c * B_LOC:(c + 1) * B_LOC] = res.results[c]["out"][:, :B_LOC]
    return h[:OUT], h[OUT:]
